# revision 16
# baseline (speedup 1.0000x reference)
"""Trainium2 Bass kernel for the EventTempRel poincare loss — gather design.

Sharding: pure data parallel over 8 NeuronCores; core m takes batch rows
[8m, 8m+8) and the aligned negatives (j-major locally); host averages the
64 per-row partial losses (the all-reduce mean).

The token-selection masks are one-hot over S, so the masked reduction
sum_s mask[s] * enc[s, :] is a gather: on device, extract each index as
dot(mask, iota) (exact for one-hot), indirect-DMA the 48 selected rows out
of the batch-local [40, S, H] token table, scale by the mask row-sum (so
all-zero masks still give the zero vector), then run the poincare tail:
  mobius_matvec(W, expmap0(x)) == expmap0(x @ W.T)   (exp/log maps cancel)
  exp(-2*artanh(d)) == (1-d)/(1+d)                   (avoids Exp/extra Ln)
ACT ops are ordered to minimize activation-table swaps (Square rides along
in every table; Sqrt/Tanh/Arctan/Ln each load once).
"""

import sys

if "/opt/trn_rl_repo" not in sys.path:
    sys.path.insert(0, "/opt/trn_rl_repo")

import numpy as np

import concourse.bacc as bacc
import concourse.bass as bass
import concourse.tile as tile
from concourse import mybir
from concourse.bass_utils import run_bass_kernel_spmd

F32 = mybir.dt.float32
I32 = mybir.dt.int32
AF = mybir.ActivationFunctionType
ALU = mybir.AluOpType

EPS = 1e-15
BND = 1.0 - 1e-7
PI_HALF = float(np.pi / 2.0)

B, S, H, D, NEG = 64, 256, 768, 64, 4
NCORES = 8
BL = B // NCORES   # 8 local batch rows
NL = BL * NEG      # 32 local negative rows
NR = BL + NL       # 40 rows in the local token table
HC = H // 128      # 6 h-chunks
NP = 2 * BL + NL   # 48 selected rows: u 0..7, v 8..15, neg 16..47 (j-major)
ND = BL + NL       # 40 distance pairs: (u,v) 0..7, (u,un_j) 8..39

U_BCAST_DMA = False  # step-0 AP DMA rejected for SBUF source; use split DMAs


def _build_nc():
    nc = bacc.Bacc(name="poincare_gather")

    allenc = nc.dram_tensor("allenc", [NR, S, H], F32, kind="ExternalInput")
    # packed per-row consts: [mask (S) | iota (S) | rowbase (1)]
    mio = nc.dram_tensor("mio", [NP, 2 * S + 1], F32, kind="ExternalInput")
    # packed: [W^T chunks (HC*D) | identity (128)]
    wid = nc.dram_tensor("wid", [128, HC * D + 128], F32, kind="ExternalInput")
    out = nc.dram_tensor("out", [BL, 1], F32, kind="ExternalOutput")

    enc2d = allenc.rearrange("r s h -> (r s) h")

    with tile.TileContext(nc) as tc:
        with (
            tc.tile_pool(name="consts", bufs=1) as consts,
            tc.tile_pool(name="work", bufs=1) as work,
            tc.tile_pool(name="stats", bufs=1) as stats,
            tc.tile_pool(name="psum", bufs=2, space="PSUM") as psp,
        ):
            sb_mio = consts.tile([NP, 2 * S + 1], F32)
            sb_wid = consts.tile([128, HC * D + 128], F32)
            nc.sync.dma_start(out=sb_mio, in_=mio[:])
            nc.scalar.dma_start(out=sb_wid, in_=wid[:])
            sb_m = sb_mio[:, 0:S]
            sb_io = sb_mio[:, S : 2 * S]
            sb_rb = sb_mio[:, 2 * S : 2 * S + 1]
            sb_wt = sb_wid[:, 0 : HC * D].rearrange("p (c d) -> p c d", c=HC)
            sb_id = sb_wid[:, HC * D : HC * D + 128]

            # ---- A: idx = dot(mask, iota) + rowbase ; msum for zero-mask ----
            prod = work.tile([NP, S], F32, tag="prod")
            nc.vector.tensor_mul(prod, sb_m, sb_io)
            sel = stats.tile([NP, 1], F32, tag="sel")
            nc.vector.reduce_sum(out=sel, in_=prod, axis=mybir.AxisListType.X)
            nc.vector.tensor_add(sel, sel, sb_rb)
            idx = stats.tile([NP, 1], I32, tag="idx")
            nc.vector.tensor_copy(out=idx, in_=sel)
            msum = stats.tile([NP, 1], F32, tag="msum")
            nc.vector.reduce_sum(out=msum, in_=sb_m, axis=mybir.AxisListType.X)

            # ---- B: gather the 48 selected token rows ----
            y = work.tile([NP, H], F32, tag="y")
            nc.gpsimd.indirect_dma_start(
                out=y[:], out_offset=None, in_=enc2d[:],
                in_offset=bass.IndirectOffsetOnAxis(ap=idx[:, :1], axis=0),
            )
            nc.vector.tensor_scalar_mul(out=y, in0=y, scalar1=msum)

            # ---- C: mraw = y @ W.T via PE transpose + contraction ----
            ut = work.tile([128, HC, NP], F32, tag="ut")
            for hcx in range(HC):
                pt = psp.tile([128, NP], F32, tag="tr")
                nc.tensor.transpose(
                    pt, y[:, hcx * 128 : (hcx + 1) * 128], sb_id[:NP, :NP]
                )
                nc.vector.tensor_copy(out=ut[:, hcx, :], in_=pt)
            pmx = psp.tile([NP, D], F32, tag="mx")
            for hcx in range(HC):
                nc.tensor.matmul(
                    pmx, ut[:, hcx, :], sb_wt[:, hcx, :],
                    start=(hcx == 0), stop=(hcx == HC - 1),
                )
            mxa = work.tile([NP, D], F32, tag="mxa")
            nc.vector.tensor_copy(out=mxa, in_=pmx)

            # ---- D: p = expmap0(mraw); |p| == tanh(|mraw|) exactly ----
            sq = work.tile([NP, D], F32, tag="sq")
            mn2 = stats.tile([NP, 1], F32, tag="mn2")
            nc.scalar.activation(out=sq, in_=mxa, func=AF.Square, accum_out=mn2)
            mnn = stats.tile([NP, 1], F32, tag="mnn")
            nc.scalar.activation(out=mnn, in_=mn2, func=AF.Sqrt)
            nc.vector.tensor_scalar_max(out=mnn, in0=mnn, scalar1=EPS)
            th = stats.tile([NP, 1], F32, tag="th")
            nc.scalar.activation(out=th, in_=mnn, func=AF.Tanh)
            f = stats.tile([NP, 1], F32, tag="f")
            nc.vector.reciprocal(out=f, in_=mnn)
            nc.vector.tensor_mul(f, th, f)
            nc.vector.tensor_scalar_mul(out=mxa, in0=mxa, scalar1=f)  # mxa := p

            # ---- E: pair tiles (v+negs contiguous; u replicated 5x) ----
            X = work.tile([ND, D], F32, tag="X")
            nc.sync.dma_start(out=X, in_=mxa[BL:NP, :])
            U = work.tile([ND, D], F32, tag="U")
            for jj in range(5):
                eng = nc.sync if jj % 2 else nc.scalar
                eng.dma_start(out=U[jj * BL : (jj + 1) * BL, :], in_=mxa[0:BL, :])

            # ---- F: cross stats (Square rides along in every ACT table) ----
            x2 = stats.tile([ND, 1], F32, tag="x2")
            sqx = work.tile([ND, D], F32, tag="sqx")
            nc.scalar.activation(out=sqx, in_=X, func=AF.Square, accum_out=x2)
            u2 = stats.tile([ND, 1], F32, tag="u2")
            squ = work.tile([ND, D], F32, tag="squ")
            nc.scalar.activation(out=squ, in_=U, func=AF.Square, accum_out=u2)
            dotp = stats.tile([ND, 1], F32, tag="dotp")
            prd = work.tile([ND, D], F32, tag="prd")
            nc.vector.tensor_mul(prd, U, X)
            nc.vector.reduce_sum(out=dotp, in_=prd, axis=mybir.AxisListType.X)
            dif = work.tile([BL, D], F32, tag="dif")
            nc.vector.tensor_sub(dif, U[0:BL, :], X[0:BL, :])
            e2 = stats.tile([BL, 1], F32, tag="e2")
            sqd = work.tile([BL, D], F32, tag="sqd")
            nc.scalar.activation(out=sqd, in_=dif, func=AF.Square, accum_out=e2)

            # ---- G: |mobius_add(-u, x)|^2 (x2_=u2, y2_=x2, xy=-dot), DVE only ----
            c1 = stats.tile([ND, 1], F32, tag="c1")
            nc.vector.tensor_scalar(
                out=c1, in0=dotp, scalar1=-2.0, scalar2=1.0, op0=ALU.mult, op1=ALU.add
            )                                     # 1 - 2dot
            dm = stats.tile([ND, 1], F32, tag="dm")
            nc.vector.tensor_mul(dm, u2, x2)
            nc.vector.tensor_add(dm, dm, c1)      # 1 - 2dot + u2*x2 (== rad for uv)
            nc.vector.tensor_scalar_max(out=dm, in0=dm, scalar1=EPS)
            nc.vector.tensor_add(c1, c1, x2)      # 1 - 2dot + x2
            c2 = stats.tile([ND, 1], F32, tag="c2")
            nc.vector.tensor_scalar(
                out=c2, in0=u2, scalar1=-1.0, scalar2=1.0, op0=ALU.mult, op1=ALU.add
            )                                     # 1 - u2
            mv = work.tile([ND, D], F32, tag="mv")
            mv2 = work.tile([ND, D], F32, tag="mv2")
            nc.vector.tensor_scalar_mul(out=mv, in0=X, scalar1=c2)
            nc.vector.tensor_scalar_mul(out=mv2, in0=U, scalar1=c1)
            nc.vector.tensor_sub(mv, mv, mv2)
            rdm = stats.tile([ND, 1], F32, tag="rdm")
            nc.vector.reciprocal(out=rdm, in_=dm)
            nc.vector.tensor_scalar_mul(out=mv, in0=mv, scalar1=rdm)
            dn2 = stats.tile([ND, 1], F32, tag="dn2")
            sqm = work.tile([ND, D], F32, tag="sqm")
            nc.scalar.activation(out=sqm, in_=mv, func=AF.Square, accum_out=dn2)

            # ---- H: Sqrt batch; den = sqrt(nv2 * e2 * rad) ----
            dn = stats.tile([ND, 1], F32, tag="dn")
            nc.scalar.activation(out=dn, in_=dn2, func=AF.Sqrt)
            dpr = stats.tile([BL, 1], F32, tag="dpr")
            nc.vector.tensor_mul(dpr, x2[0:BL, :], e2)
            nc.vector.tensor_mul(dpr, dpr, dm[0:BL, :])
            den = stats.tile([BL, 1], F32, tag="den")
            nc.scalar.activation(out=den, in_=dpr, func=AF.Sqrt)
            nc.vector.tensor_scalar_max(out=den, in0=den, scalar1=EPS)
            nc.vector.tensor_scalar_min(out=dn, in0=dn, scalar1=BND)

            # angles: cos = (dot*(1+x2) - x2*(1+u2)) / den, clipped
            t1 = stats.tile([BL, 1], F32, tag="t1")
            nc.vector.tensor_scalar_add(out=t1, in0=x2[0:BL, :], scalar1=1.0)
            nc.vector.tensor_mul(t1, dotp[0:BL, :], t1)
            t2 = stats.tile([BL, 1], F32, tag="t2")
            nc.vector.tensor_scalar_add(out=t2, in0=u2[0:BL, :], scalar1=1.0)
            nc.vector.tensor_mul(t2, x2[0:BL, :], t2)
            cosn = stats.tile([BL, 1], F32, tag="cosn")
            nc.vector.tensor_sub(cosn, t1, t2)
            rden = stats.tile([BL, 1], F32, tag="rden")
            nc.vector.reciprocal(out=rden, in_=den)
            nc.vector.tensor_mul(cosn, cosn, rden)
            nc.vector.tensor_scalar(
                out=cosn, in0=cosn, scalar1=-BND, scalar2=BND, op0=ALU.max, op1=ALU.min
            )
            c2t = stats.tile([BL, 1], F32, tag="c2t")
            nc.vector.tensor_mul(c2t, cosn, cosn)
            nc.vector.tensor_scalar(
                out=c2t, in0=c2t, scalar1=-1.0, scalar2=1.0, op0=ALU.mult, op1=ALU.add
            )
            sc2 = stats.tile([BL, 1], F32, tag="sc2")
            nc.scalar.activation(out=sc2, in_=c2t, func=AF.Sqrt)
            rsc = stats.tile([BL, 1], F32, tag="rsc")
            nc.vector.reciprocal(out=rsc, in_=sc2)
            aarg = stats.tile([BL, 1], F32, tag="aarg")
            nc.vector.tensor_mul(aarg, cosn, rsc)

            # ---- I: angles = pi/2 - arctan(aarg) ----
            atv = stats.tile([BL, 1], F32, tag="atv")
            nc.scalar.activation(out=atv, in_=aarg, func=AF.Arctan)
            ang = stats.tile([BL, 1], F32, tag="ang")
            nc.vector.tensor_scalar(
                out=ang, in0=atv, scalar1=PI_HALF, scalar2=-1.0,
                op0=ALU.subtract, op1=ALU.mult,
            )

            # ---- J: exp(-dsq) = (1-dn)/(1+dn); dsq = ln((1+dn)/(1-dn)) ----
            opd = stats.tile([ND, 1], F32, tag="opd")
            nc.vector.tensor_scalar_add(out=opd, in0=dn, scalar1=1.0)
            omd = stats.tile([ND, 1], F32, tag="omd")
            nc.vector.tensor_scalar(
                out=omd, in0=dn, scalar1=-1.0, scalar2=1.0, op0=ALU.mult, op1=ALU.add
            )
            ropd = stats.tile([ND, 1], F32, tag="ropd")
            nc.vector.reciprocal(out=ropd, in_=opd)
            en = stats.tile([ND, 1], F32, tag="en")
            nc.vector.tensor_mul(en, omd, ropd)           # exp(-dsq), all 40 pairs
            romd = stats.tile([BL, 1], F32, tag="romd")
            nc.vector.reciprocal(out=romd, in_=omd[0:BL, :])
            ratio = stats.tile([BL, 1], F32, tag="ratio")
            nc.vector.tensor_mul(ratio, opd[0:BL, :], romd)
            dsq = stats.tile([BL, 1], F32, tag="dsq")
            nc.scalar.activation(out=dsq, in_=ratio, func=AF.Ln)

            # Z1 gather (neg pair rows BL + j*BL + b) and final loss rows
            en84 = stats.tile([BL, NEG], F32, tag="en84")
            for jj in range(NEG):
                eng = nc.sync if jj % 2 else nc.scalar
                eng.dma_start(
                    out=en84[:, jj : jj + 1],
                    in_=en[BL + jj * BL : BL + (jj + 1) * BL, :],
                )
            z1 = stats.tile([BL, 1], F32, tag="z1")
            nc.vector.reduce_sum(out=z1, in_=en84, axis=mybir.AxisListType.X)
            nc.vector.tensor_add(z1, z1, en[0:BL, :])
            lnz = stats.tile([BL, 1], F32, tag="lnz")
            nc.scalar.activation(out=lnz, in_=z1, func=AF.Ln)
            lrow = stats.tile([BL, 1], F32, tag="lrow")
            nc.vector.tensor_add(lrow, lnz, dsq)
            nc.vector.tensor_add(lrow, lrow, ang)
            nc.sync.dma_start(out=out[:], in_=lrow)

    nc.compile()
    return nc


_NC_CACHE = None


def _get_nc():
    global _NC_CACHE
    if _NC_CACHE is None:
        _NC_CACHE = _build_nc()
    return _NC_CACHE


def _prep_core_inputs(encoded, n_encoded, mask1, mask2, mask_u_neg, W):
    m1 = np.ascontiguousarray(mask1.reshape(B, S), dtype=np.float32)
    m2 = np.ascontiguousarray(mask2.reshape(B, S), dtype=np.float32)
    mnr = np.ascontiguousarray(mask_u_neg.reshape(B * NEG, S), dtype=np.float32)
    wid = np.zeros((128, HC * D + 128), dtype=np.float32)
    wid[:, 0 : HC * D] = (
        W.astype(np.float32).T.reshape(HC, 128, D).transpose(1, 0, 2).reshape(128, -1)
    )
    wid[:, HC * D :] = np.eye(128, dtype=np.float32)
    iota = np.arange(S, dtype=np.float32)
    # selected-row -> local token-table row: u_b -> b, v_b -> b, neg (j-major) -> 8+jl
    rowbase = (
        np.concatenate([np.arange(BL), np.arange(BL), BL + np.arange(NL)]).astype(
            np.float32
        )
        * S
    )
    in_maps = []
    for m in range(NCORES):
        b0, j0 = m * BL, m * NL
        nenc_l = (
            n_encoded[j0 : j0 + NL]
            .reshape(BL, NEG, S, H)
            .transpose(1, 0, 2, 3)
            .reshape(NL, S, H)
        )
        allenc = np.concatenate(
            [np.asarray(encoded[b0 : b0 + BL], dtype=np.float32), nenc_l], axis=0
        )
        mn_l = (
            mnr[j0 : j0 + NL].reshape(BL, NEG, S).transpose(1, 0, 2).reshape(NL, S)
        )
        mall = np.concatenate([m1[b0 : b0 + BL], m2[b0 : b0 + BL], mn_l], axis=0)
        mio = np.concatenate(
            [mall, np.broadcast_to(iota, (NP, S)), rowbase[:, None]], axis=1
        )
        in_maps.append(
            {
                "allenc": np.ascontiguousarray(allenc),
                "mio": np.ascontiguousarray(mio, dtype=np.float32),
                "wid": wid,
            }
        )
    return in_maps


def kernel(encoded, n_encoded, mask1, mask2, mask_u_neg, W):
    nc = _get_nc()
    in_maps = _prep_core_inputs(encoded, n_encoded, mask1, mask2, mask_u_neg, W)
    res = run_bass_kernel_spmd(nc, in_maps, core_ids=list(range(NCORES)))
    rows = np.concatenate([r["out"][:, 0] for r in res.results])
    return np.float32(rows.mean())


# revision 17
# speedup vs baseline: 1.2562x; 1.2562x over previous
"""Trainium2 Bass kernel for the EventTempRel poincare loss — gather design.

Sharding: pure data parallel over 8 NeuronCores; core m takes batch rows
[8m, 8m+8) and the aligned negatives (j-major locally); host averages the
64 per-row partial losses (the all-reduce mean).

The token-selection masks are one-hot over S, so the masked reduction
sum_s mask[s] * enc[s, :] is a gather: on device, extract each index as
dot(mask, iota) (exact for one-hot), indirect-DMA the 48 selected rows out
of the batch-local [40, S, H] token table, scale by the mask row-sum (so
all-zero masks still give the zero vector), then run the poincare tail:
  mobius_matvec(W, expmap0(x)) == expmap0(x @ W.T)   (exp/log maps cancel)
  exp(-2*artanh(d)) == (1-d)/(1+d)                   (avoids Exp/extra Ln)
ACT ops are ordered to minimize activation-table swaps (Square rides along
in every table; Sqrt/Tanh/Arctan/Ln each load once).
"""

import sys

if "/opt/trn_rl_repo" not in sys.path:
    sys.path.insert(0, "/opt/trn_rl_repo")

import numpy as np

import concourse.bacc as bacc
import concourse.bass as bass
import concourse.tile as tile
from concourse import mybir
from concourse.bass_utils import run_bass_kernel_spmd

F32 = mybir.dt.float32
I32 = mybir.dt.int32
AF = mybir.ActivationFunctionType
ALU = mybir.AluOpType

EPS = 1e-15
BND = 1.0 - 1e-7
PI_HALF = float(np.pi / 2.0)

B, S, H, D, NEG = 64, 256, 768, 64, 4
NCORES = 8
BL = B // NCORES   # 8 local batch rows
NL = BL * NEG      # 32 local negative rows
NR = BL + NL       # 40 rows in the local token table
HC = H // 128      # 6 h-chunks
NP = 2 * BL + NL   # 48 selected rows: u 0..7, v 8..15, neg 16..47 (j-major)
ND = BL + NL       # 40 distance pairs: (u,v) 0..7, (u,un_j) 8..39

U_BCAST_DMA = False  # step-0 AP DMA rejected for SBUF source; use split DMAs


def _build_nc():
    nc = bacc.Bacc(name="poincare_gather")

    allenc = nc.dram_tensor("allenc", [NR, S, H], F32, kind="ExternalInput")
    # packed per-row consts: [mask (S) | iota (S) | rowbase (1)]
    mio = nc.dram_tensor("mio", [NP, 2 * S + 1], F32, kind="ExternalInput")
    # packed: [W^T chunks (HC*D) | identity (128)]
    wid = nc.dram_tensor("wid", [128, HC * D + 128], F32, kind="ExternalInput")
    out = nc.dram_tensor("out", [BL, 1], F32, kind="ExternalOutput")

    enc2d = allenc.rearrange("r s h -> (r s) h")

    with tile.TileContext(nc) as tc:
        with (
            tc.tile_pool(name="consts", bufs=1) as consts,
            tc.tile_pool(name="work", bufs=1) as work,
            tc.tile_pool(name="stats", bufs=1) as stats,
            tc.tile_pool(name="psum", bufs=2, space="PSUM") as psp,
        ):
            sb_mio = consts.tile([NP, 2 * S + 1], F32)
            sb_wid = consts.tile([128, HC * D + 128], F32)
            nc.sync.dma_start(out=sb_mio, in_=mio[:])
            nc.scalar.dma_start(out=sb_wid, in_=wid[:])
            sb_m = sb_mio[:, 0:S]
            sb_io = sb_mio[:, S : 2 * S]
            sb_rb = sb_mio[:, 2 * S : 2 * S + 1]
            sb_wt = sb_wid[:, 0 : HC * D].rearrange("p (c d) -> p c d", c=HC)
            sb_id = sb_wid[:, HC * D : HC * D + 128]

            # ---- A: idx = dot(mask, iota) + rowbase ; msum for zero-mask ----
            prod = work.tile([NP, S], F32, tag="prod")
            nc.vector.tensor_mul(prod, sb_m, sb_io)
            sel = stats.tile([NP, 1], F32, tag="sel")
            nc.vector.reduce_sum(out=sel, in_=prod, axis=mybir.AxisListType.X)
            nc.vector.tensor_add(sel, sel, sb_rb)
            idx = stats.tile([NP, 1], I32, tag="idx")
            nc.vector.tensor_copy(out=idx, in_=sel)
            msum = stats.tile([NP, 1], F32, tag="msum")
            msc = work.tile([NP, S], F32, tag="msc")
            nc.scalar.activation(out=msc, in_=sb_m, func=AF.Identity, accum_out=msum)
            # preload the sqrt ACT table while waiting on the gather
            dum = stats.tile([NP, 1], F32, tag="dum")
            nc.scalar.activation(out=dum, in_=msum, func=AF.Sqrt)

            # ---- B: gather the 48 selected token rows ----
            y = work.tile([NP, H], F32, tag="y")
            nc.gpsimd.indirect_dma_start(
                out=y[:], out_offset=None, in_=enc2d[:],
                in_offset=bass.IndirectOffsetOnAxis(ap=idx[:, :1], axis=0),
            )

            # ---- C: mraw = y @ W.T via PE transpose + contraction ----
            ut = work.tile([128, HC, NP], F32, tag="ut")
            for hcx in range(HC):
                pt = psp.tile([128, NP], F32, tag="tr")
                nc.tensor.transpose(
                    pt, y[:, hcx * 128 : (hcx + 1) * 128], sb_id[:NP, :NP]
                )
                nc.vector.tensor_copy(out=ut[:, hcx, :], in_=pt)
            pmx = psp.tile([NP, D], F32, tag="mx")
            for hcx in range(HC):
                nc.tensor.matmul(
                    pmx, ut[:, hcx, :], sb_wt[:, hcx, :],
                    start=(hcx == 0), stop=(hcx == HC - 1),
                )
            mxa = work.tile([NP, D], F32, tag="mxa")
            nc.vector.tensor_copy(out=mxa, in_=pmx)
            nc.vector.tensor_scalar_mul(out=mxa, in0=mxa, scalar1=msum)

            # ---- D: p = expmap0(mraw); |p| == tanh(|mraw|) exactly ----
            sq = work.tile([NP, D], F32, tag="sq")
            mn2 = stats.tile([NP, 1], F32, tag="mn2")
            nc.scalar.activation(out=sq, in_=mxa, func=AF.Square, accum_out=mn2)
            mnn = stats.tile([NP, 1], F32, tag="mnn")
            nc.scalar.activation(out=mnn, in_=mn2, func=AF.Sqrt)
            nc.vector.tensor_scalar_max(out=mnn, in0=mnn, scalar1=EPS)
            th = stats.tile([NP, 1], F32, tag="th")
            nc.scalar.activation(out=th, in_=mnn, func=AF.Tanh)
            f = stats.tile([NP, 1], F32, tag="f")
            nc.vector.reciprocal(out=f, in_=mnn)
            nc.vector.tensor_mul(f, th, f)
            nc.vector.tensor_scalar_mul(out=mxa, in0=mxa, scalar1=f)  # mxa := p
            dum2 = stats.tile([NP, 1], F32, tag="dum2")
            nc.scalar.activation(out=dum2, in_=th, func=AF.Sqrt)  # re-arm sqrt table

            # ---- E: pair tiles (v+negs contiguous; u replicated 5x) ----
            X = work.tile([ND, D], F32, tag="X")
            nc.sync.dma_start(out=X, in_=mxa[BL:NP, :])
            U = work.tile([ND, D], F32, tag="U")
            for jj in range(5):
                eng = nc.sync if jj % 2 else nc.gpsimd
                eng.dma_start(out=U[jj * BL : (jj + 1) * BL, :], in_=mxa[0:BL, :])

            # ---- F: cross stats (Square rides along in every ACT table) ----
            x2 = stats.tile([ND, 1], F32, tag="x2")
            sqx = work.tile([ND, D], F32, tag="sqx")
            nc.scalar.activation(out=sqx, in_=X, func=AF.Square, accum_out=x2)
            u2 = stats.tile([ND, 1], F32, tag="u2")
            squ = work.tile([ND, D], F32, tag="squ")
            nc.scalar.activation(out=squ, in_=U, func=AF.Square, accum_out=u2)
            dotp = stats.tile([ND, 1], F32, tag="dotp")
            prd = work.tile([ND, D], F32, tag="prd")
            nc.vector.tensor_mul(prd, U, X)
            nc.vector.reduce_sum(out=dotp, in_=prd, axis=mybir.AxisListType.X)
            dif = work.tile([BL, D], F32, tag="dif")
            nc.vector.tensor_sub(dif, U[0:BL, :], X[0:BL, :])
            e2 = stats.tile([BL, 1], F32, tag="e2")
            sqd = work.tile([BL, D], F32, tag="sqd")
            nc.scalar.activation(out=sqd, in_=dif, func=AF.Square, accum_out=e2)

            # ---- G: |mobius_add(-u, x)|^2 (x2_=u2, y2_=x2, xy=-dot), DVE only ----
            c1 = stats.tile([ND, 1], F32, tag="c1")
            nc.vector.tensor_scalar(
                out=c1, in0=dotp, scalar1=-2.0, scalar2=1.0, op0=ALU.mult, op1=ALU.add
            )                                     # 1 - 2dot
            dm = stats.tile([ND, 1], F32, tag="dm")
            nc.vector.tensor_mul(dm, u2, x2)
            nc.vector.tensor_add(dm, dm, c1)      # 1 - 2dot + u2*x2 (== rad for uv)
            nc.vector.tensor_scalar_max(out=dm, in0=dm, scalar1=EPS)
            nc.vector.tensor_add(c1, c1, x2)      # 1 - 2dot + x2
            c2 = stats.tile([ND, 1], F32, tag="c2")
            nc.vector.tensor_scalar(
                out=c2, in0=u2, scalar1=-1.0, scalar2=1.0, op0=ALU.mult, op1=ALU.add
            )                                     # 1 - u2
            mv = work.tile([ND, D], F32, tag="mv")
            mv2 = work.tile([ND, D], F32, tag="mv2")
            nc.vector.tensor_scalar_mul(out=mv, in0=X, scalar1=c2)
            nc.vector.tensor_scalar_mul(out=mv2, in0=U, scalar1=c1)
            nc.vector.tensor_sub(mv, mv, mv2)
            rdm = stats.tile([ND, 1], F32, tag="rdm")
            nc.vector.reciprocal(out=rdm, in_=dm)
            nc.vector.tensor_scalar_mul(out=mv, in0=mv, scalar1=rdm)
            dn2 = stats.tile([ND, 1], F32, tag="dn2")
            sqm = work.tile([ND, D], F32, tag="sqm")
            nc.scalar.activation(out=sqm, in_=mv, func=AF.Square, accum_out=dn2)

            # ---- H: Sqrt batch; den = sqrt(nv2 * e2 * rad) ----
            dn = stats.tile([ND, 1], F32, tag="dn")
            nc.scalar.activation(out=dn, in_=dn2, func=AF.Sqrt)
            dpr = stats.tile([BL, 1], F32, tag="dpr")
            nc.vector.tensor_mul(dpr, x2[0:BL, :], e2)
            nc.vector.tensor_mul(dpr, dpr, dm[0:BL, :])
            den = stats.tile([BL, 1], F32, tag="den")
            nc.scalar.activation(out=den, in_=dpr, func=AF.Sqrt)
            nc.vector.tensor_scalar_max(out=den, in0=den, scalar1=EPS)
            nc.vector.tensor_scalar_min(out=dn, in0=dn, scalar1=BND)

            # angles: cos = (dot*(1+x2) - x2*(1+u2)) / den, clipped
            t1 = stats.tile([BL, 1], F32, tag="t1")
            nc.vector.tensor_scalar_add(out=t1, in0=x2[0:BL, :], scalar1=1.0)
            nc.vector.tensor_mul(t1, dotp[0:BL, :], t1)
            t2 = stats.tile([BL, 1], F32, tag="t2")
            nc.vector.tensor_scalar_add(out=t2, in0=u2[0:BL, :], scalar1=1.0)
            nc.vector.tensor_mul(t2, x2[0:BL, :], t2)
            cosn = stats.tile([BL, 1], F32, tag="cosn")
            nc.vector.tensor_sub(cosn, t1, t2)
            rden = stats.tile([BL, 1], F32, tag="rden")
            nc.vector.reciprocal(out=rden, in_=den)
            nc.vector.tensor_mul(cosn, cosn, rden)
            nc.vector.tensor_scalar(
                out=cosn, in0=cosn, scalar1=-BND, scalar2=BND, op0=ALU.max, op1=ALU.min
            )
            c2t = stats.tile([BL, 1], F32, tag="c2t")
            nc.vector.tensor_mul(c2t, cosn, cosn)
            nc.vector.tensor_scalar(
                out=c2t, in0=c2t, scalar1=-1.0, scalar2=1.0, op0=ALU.mult, op1=ALU.add
            )
            sc2 = stats.tile([BL, 1], F32, tag="sc2")
            nc.scalar.activation(out=sc2, in_=c2t, func=AF.Sqrt)
            rsc = stats.tile([BL, 1], F32, tag="rsc")
            nc.vector.reciprocal(out=rsc, in_=sc2)
            aarg = stats.tile([BL, 1], F32, tag="aarg")
            nc.vector.tensor_mul(aarg, cosn, rsc)

            # ---- I: angles = pi/2 - arctan(aarg) ----
            atv = stats.tile([BL, 1], F32, tag="atv")
            nc.scalar.activation(out=atv, in_=aarg, func=AF.Arctan)
            ang = stats.tile([BL, 1], F32, tag="ang")
            nc.vector.tensor_scalar(
                out=ang, in0=atv, scalar1=PI_HALF, scalar2=-1.0,
                op0=ALU.subtract, op1=ALU.mult,
            )

            # ---- J: exp(-dsq) = (1-dn)/(1+dn); dsq = ln((1+dn)/(1-dn)) ----
            opd = stats.tile([ND, 1], F32, tag="opd")
            nc.vector.tensor_scalar_add(out=opd, in0=dn, scalar1=1.0)
            omd = stats.tile([ND, 1], F32, tag="omd")
            nc.vector.tensor_scalar(
                out=omd, in0=dn, scalar1=-1.0, scalar2=1.0, op0=ALU.mult, op1=ALU.add
            )
            ropd = stats.tile([ND, 1], F32, tag="ropd")
            nc.vector.reciprocal(out=ropd, in_=opd)
            en = stats.tile([ND, 1], F32, tag="en")
            nc.vector.tensor_mul(en, omd, ropd)           # exp(-dsq), all 40 pairs
            romd = stats.tile([BL, 1], F32, tag="romd")
            nc.vector.reciprocal(out=romd, in_=omd[0:BL, :])
            ratio = stats.tile([BL, 1], F32, tag="ratio")
            nc.vector.tensor_mul(ratio, opd[0:BL, :], romd)
            dsq = stats.tile([BL, 1], F32, tag="dsq")
            nc.scalar.activation(out=dsq, in_=ratio, func=AF.Ln)

            # Z1 gather (neg pair rows BL + j*BL + b) and final loss rows
            en84 = stats.tile([BL, NEG], F32, tag="en84")
            for jj in range(NEG):
                eng = nc.sync if jj % 2 else nc.gpsimd
                eng.dma_start(
                    out=en84[:, jj : jj + 1],
                    in_=en[BL + jj * BL : BL + (jj + 1) * BL, :],
                )
            z1 = stats.tile([BL, 1], F32, tag="z1")
            nc.vector.reduce_sum(out=z1, in_=en84, axis=mybir.AxisListType.X)
            nc.vector.tensor_add(z1, z1, en[0:BL, :])
            lnz = stats.tile([BL, 1], F32, tag="lnz")
            nc.scalar.activation(out=lnz, in_=z1, func=AF.Ln)
            lrow = stats.tile([BL, 1], F32, tag="lrow")
            nc.vector.tensor_add(lrow, lnz, dsq)
            nc.vector.tensor_add(lrow, lrow, ang)
            nc.sync.dma_start(out=out[:], in_=lrow)

    nc.compile()
    return nc


_NC_CACHE = None


def _get_nc():
    global _NC_CACHE
    if _NC_CACHE is None:
        _NC_CACHE = _build_nc()
    return _NC_CACHE


def _prep_core_inputs(encoded, n_encoded, mask1, mask2, mask_u_neg, W):
    m1 = np.ascontiguousarray(mask1.reshape(B, S), dtype=np.float32)
    m2 = np.ascontiguousarray(mask2.reshape(B, S), dtype=np.float32)
    mnr = np.ascontiguousarray(mask_u_neg.reshape(B * NEG, S), dtype=np.float32)
    wid = np.zeros((128, HC * D + 128), dtype=np.float32)
    wid[:, 0 : HC * D] = (
        W.astype(np.float32).T.reshape(HC, 128, D).transpose(1, 0, 2).reshape(128, -1)
    )
    wid[:, HC * D :] = np.eye(128, dtype=np.float32)
    iota = np.arange(S, dtype=np.float32)
    # selected-row -> local token-table row: u_b -> b, v_b -> b, neg (j-major) -> 8+jl
    rowbase = (
        np.concatenate([np.arange(BL), np.arange(BL), BL + np.arange(NL)]).astype(
            np.float32
        )
        * S
    )
    in_maps = []
    for m in range(NCORES):
        b0, j0 = m * BL, m * NL
        nenc_l = (
            n_encoded[j0 : j0 + NL]
            .reshape(BL, NEG, S, H)
            .transpose(1, 0, 2, 3)
            .reshape(NL, S, H)
        )
        allenc = np.concatenate(
            [np.asarray(encoded[b0 : b0 + BL], dtype=np.float32), nenc_l], axis=0
        )
        mn_l = (
            mnr[j0 : j0 + NL].reshape(BL, NEG, S).transpose(1, 0, 2).reshape(NL, S)
        )
        mall = np.concatenate([m1[b0 : b0 + BL], m2[b0 : b0 + BL], mn_l], axis=0)
        mio = np.concatenate(
            [mall, np.broadcast_to(iota, (NP, S)), rowbase[:, None]], axis=1
        )
        in_maps.append(
            {
                "allenc": np.ascontiguousarray(allenc),
                "mio": np.ascontiguousarray(mio, dtype=np.float32),
                "wid": wid,
            }
        )
    return in_maps


def kernel(encoded, n_encoded, mask1, mask2, mask_u_neg, W):
    nc = _get_nc()
    in_maps = _prep_core_inputs(encoded, n_encoded, mask1, mask2, mask_u_neg, W)
    res = run_bass_kernel_spmd(nc, in_maps, core_ids=list(range(NCORES)))
    rows = np.concatenate([r["out"][:, 0] for r in res.results])
    return np.float32(rows.mean())


# revision 20
# speedup vs baseline: 1.3929x; 1.1089x over previous
"""Trainium2 Bass kernel for the EventTempRel poincare loss — gather design.

Sharding: pure data parallel over 8 NeuronCores; core m takes batch rows
[8m, 8m+8) and the aligned negatives (j-major locally); host averages the
64 per-row partial losses (the all-reduce mean).

The token-selection masks are one-hot over S, so the masked reduction
sum_s mask[s] * enc[s, :] is a gather: on device, extract each index as
dot(mask, iota) (exact for one-hot), indirect-DMA the 48 selected rows out
of the batch-local [40, S, H] token table, scale by the mask row-sum (so
all-zero masks still give the zero vector), then run the poincare tail:
  mobius_matvec(W, expmap0(x)) == expmap0(x @ W.T)   (exp/log maps cancel)
  exp(-2*artanh(d)) == (1-d)/(1+d)                   (avoids Exp/extra Ln)
ACT ops are ordered to minimize activation-table swaps (Square rides along
in every table; Sqrt/Tanh/Arctan/Ln each load once).
"""

import sys

if "/opt/trn_rl_repo" not in sys.path:
    sys.path.insert(0, "/opt/trn_rl_repo")

import numpy as np

import concourse.bacc as bacc
import concourse.bass as bass
import concourse.tile as tile
from concourse import mybir
from concourse.bass_utils import run_bass_kernel_spmd

F32 = mybir.dt.float32
I32 = mybir.dt.int32
AF = mybir.ActivationFunctionType
ALU = mybir.AluOpType

EPS = 1e-15
BND = 1.0 - 1e-7
PI_HALF = float(np.pi / 2.0)

B, S, H, D, NEG = 64, 256, 768, 64, 4
NCORES = 8
BL = B // NCORES   # 8 local batch rows
NL = BL * NEG      # 32 local negative rows
NR = BL + NL       # 40 rows in the local token table
HC = H // 128      # 6 h-chunks
NP = 2 * BL + NL   # 48 selected rows: u 0..7, v 8..15, neg 16..47 (j-major)
ND = BL + NL       # 40 distance pairs: (u,v) 0..7, (u,un_j) 8..39

U_BCAST_DMA = False  # step-0 AP DMA rejected for SBUF source; use split DMAs


def _build_nc():
    nc = bacc.Bacc(name="poincare_gather")

    allenc = nc.dram_tensor("allenc", [NR, S, H], F32, kind="ExternalInput")
    # packed per-row consts: [mask (S) | iota (S) | rowbase (1)]
    mio = nc.dram_tensor("mio", [NP, 2 * S + 1], F32, kind="ExternalInput")
    # packed: [W^T chunks (HC*D) | identity (128)]
    wid = nc.dram_tensor("wid", [128, HC * D + 128], F32, kind="ExternalInput")
    out = nc.dram_tensor("out", [BL, 1], F32, kind="ExternalOutput")

    enc2d = allenc.rearrange("r s h -> (r s) h")

    with tile.TileContext(nc) as tc:
        with (
            tc.tile_pool(name="consts", bufs=1) as consts,
            tc.tile_pool(name="work", bufs=1) as work,
            tc.tile_pool(name="stats", bufs=1) as stats,
            tc.tile_pool(name="psum", bufs=2, space="PSUM") as psp,
        ):
            sb_mio = consts.tile([NP, 2 * S + 1], F32)
            sb_wid = consts.tile([128, HC * D + 128], F32)
            nc.sync.dma_start(out=sb_mio[:, 0:S], in_=mio[:, 0:S])
            nc.scalar.dma_start(out=sb_mio[:, S : 2 * S + 1], in_=mio[:, S : 2 * S + 1])
            nc.scalar.dma_start(out=sb_wid, in_=wid[:])
            sb_m = sb_mio[:, 0:S]
            sb_io = sb_mio[:, S : 2 * S]
            sb_rb = sb_mio[:, 2 * S : 2 * S + 1]
            sb_wt = sb_wid[:, 0 : HC * D].rearrange("p (c d) -> p c d", c=HC)
            sb_id = sb_wid[:, HC * D : HC * D + 128]

            # ---- A: idx = dot(mask, iota) + rowbase ; msum for zero-mask ----
            prod = work.tile([NP, S], F32, tag="prod")
            nc.vector.tensor_mul(prod, sb_m, sb_io)
            sel = stats.tile([NP, 1], F32, tag="sel")
            nc.vector.reduce_sum(out=sel, in_=prod, axis=mybir.AxisListType.X)
            nc.vector.tensor_add(sel, sel, sb_rb)
            idx = stats.tile([NP, 1], I32, tag="idx")
            nc.vector.tensor_copy(out=idx, in_=sel)
            msum = stats.tile([NP, 1], F32, tag="msum")
            msc = work.tile([NP, S], F32, tag="msc")
            nc.scalar.activation(out=msc, in_=sb_m, func=AF.Identity, accum_out=msum)
            # preload the sqrt ACT table while waiting on the gather
            dum = stats.tile([NP, 1], F32, tag="dum")
            nc.scalar.activation(out=dum, in_=msum, func=AF.Sqrt)

            # ---- B: gather the 48 selected token rows (split at partition 32
            # so the first chunk's transposes overlap the second's latency) ----
            y = work.tile([NP, H], F32, tag="y")
            nc.gpsimd.indirect_dma_start(
                out=y[:], out_offset=None, in_=enc2d[:],
                in_offset=bass.IndirectOffsetOnAxis(ap=idx[:, :1], axis=0),
            )

            # ---- C: mraw = y @ W.T via PE transpose + contraction ----
            ut = work.tile([128, HC, NP], F32, tag="ut")
            for hcx in range(HC):
                pt = psp.tile([128, NP], F32, tag="tr")
                nc.tensor.transpose(
                    pt, y[:, hcx * 128 : (hcx + 1) * 128], sb_id[:NP, :NP]
                )
                nc.vector.tensor_copy(out=ut[:, hcx, :], in_=pt)
            pmx = psp.tile([NP, D], F32, tag="mx")
            for hcx in range(HC):
                nc.tensor.matmul(
                    pmx, ut[:, hcx, :], sb_wt[:, hcx, :],
                    start=(hcx == 0), stop=(hcx == HC - 1),
                )
            mxa = work.tile([NP, D], F32, tag="mxa")
            nc.vector.tensor_copy(out=mxa, in_=pmx)
            nc.vector.tensor_scalar_mul(out=mxa, in0=mxa, scalar1=msum)

            # ---- D: p = expmap0(mraw); |p| == tanh(|mraw|) exactly ----
            sq = work.tile([NP, D], F32, tag="sq")
            mn2 = stats.tile([NP, 1], F32, tag="mn2")
            nc.scalar.activation(out=sq, in_=mxa, func=AF.Square, accum_out=mn2)
            mnn = stats.tile([NP, 1], F32, tag="mnn")
            nc.scalar.activation(out=mnn, in_=mn2, func=AF.Sqrt)
            nc.vector.tensor_scalar_max(out=mnn, in0=mnn, scalar1=EPS)
            th = stats.tile([NP, 1], F32, tag="th")
            nc.scalar.activation(out=th, in_=mnn, func=AF.Tanh)
            f = stats.tile([NP, 1], F32, tag="f")
            nc.vector.reciprocal(out=f, in_=mnn)
            nc.vector.tensor_mul(f, th, f)
            nc.vector.tensor_scalar_mul(out=mxa, in0=mxa, scalar1=f)  # mxa := p
            dum2 = stats.tile([NP, 1], F32, tag="dum2")
            nc.scalar.activation(out=dum2, in_=th, func=AF.Sqrt)  # re-arm sqrt table

            # ---- E: pair tiles (v+negs contiguous; u replicated 5x) ----
            X = work.tile([ND, D], F32, tag="X")
            nc.sync.dma_start(out=X, in_=mxa[BL:NP, :])
            U = work.tile([ND, D], F32, tag="U")
            for jj in range(5):
                eng = nc.sync if jj % 2 else nc.gpsimd
                eng.dma_start(out=U[jj * BL : (jj + 1) * BL, :], in_=mxa[0:BL, :])

            # ---- F: cross stats (Square rides along in every ACT table) ----
            x2 = stats.tile([ND, 1], F32, tag="x2")
            sqx = work.tile([ND, D], F32, tag="sqx")
            nc.scalar.activation(out=sqx, in_=X, func=AF.Square, accum_out=x2)
            u2 = stats.tile([ND, 1], F32, tag="u2")
            squ = work.tile([ND, D], F32, tag="squ")
            nc.scalar.activation(out=squ, in_=U, func=AF.Square, accum_out=u2)
            dotp = stats.tile([ND, 1], F32, tag="dotp")
            prd = work.tile([ND, D], F32, tag="prd")
            nc.vector.tensor_mul(prd, U, X)
            nc.vector.reduce_sum(out=dotp, in_=prd, axis=mybir.AxisListType.X)
            dif = work.tile([BL, D], F32, tag="dif")
            nc.vector.tensor_sub(dif, U[0:BL, :], X[0:BL, :])
            e2 = stats.tile([BL, 1], F32, tag="e2")
            sqd = work.tile([BL, D], F32, tag="sqd")
            nc.scalar.activation(out=sqd, in_=dif, func=AF.Square, accum_out=e2)

            # ---- G: |mobius_add(-u, x)|^2 (x2_=u2, y2_=x2, xy=-dot), DVE only ----
            c1 = stats.tile([ND, 1], F32, tag="c1")
            nc.vector.tensor_scalar(
                out=c1, in0=dotp, scalar1=-2.0, scalar2=1.0, op0=ALU.mult, op1=ALU.add
            )                                     # 1 - 2dot
            dm = stats.tile([ND, 1], F32, tag="dm")
            nc.vector.tensor_mul(dm, u2, x2)
            nc.vector.tensor_add(dm, dm, c1)      # 1 - 2dot + u2*x2 (== rad for uv)
            nc.vector.tensor_scalar_max(out=dm, in0=dm, scalar1=EPS)
            nc.vector.tensor_add(c1, c1, x2)      # 1 - 2dot + x2
            c2 = stats.tile([ND, 1], F32, tag="c2")
            nc.vector.tensor_scalar(
                out=c2, in0=u2, scalar1=-1.0, scalar2=1.0, op0=ALU.mult, op1=ALU.add
            )                                     # 1 - u2
            mv = work.tile([ND, D], F32, tag="mv")
            mv2 = work.tile([ND, D], F32, tag="mv2")
            nc.vector.tensor_scalar_mul(out=mv, in0=X, scalar1=c2)
            nc.vector.tensor_scalar_mul(out=mv2, in0=U, scalar1=c1)
            nc.vector.tensor_sub(mv, mv, mv2)
            rdm = stats.tile([ND, 1], F32, tag="rdm")
            nc.vector.reciprocal(out=rdm, in_=dm)
            nc.vector.tensor_scalar_mul(out=mv, in0=mv, scalar1=rdm)
            dn2 = stats.tile([ND, 1], F32, tag="dn2")
            sqm = work.tile([ND, D], F32, tag="sqm")
            nc.scalar.activation(out=sqm, in_=mv, func=AF.Square, accum_out=dn2)

            # ---- H: Sqrt batch; den = sqrt(nv2 * e2 * rad) ----
            dn = stats.tile([ND, 1], F32, tag="dn")
            nc.scalar.activation(out=dn, in_=dn2, func=AF.Sqrt)
            dpr = stats.tile([BL, 1], F32, tag="dpr")
            nc.vector.tensor_mul(dpr, x2[0:BL, :], e2)
            nc.vector.tensor_mul(dpr, dpr, dm[0:BL, :])
            den = stats.tile([BL, 1], F32, tag="den")
            nc.scalar.activation(out=den, in_=dpr, func=AF.Sqrt)
            nc.vector.tensor_scalar_max(out=den, in0=den, scalar1=EPS)
            nc.vector.tensor_scalar_min(out=dn, in0=dn, scalar1=BND)

            # angles: cos = (dot*(1+x2) - x2*(1+u2)) / den, clipped
            t1 = stats.tile([BL, 1], F32, tag="t1")
            nc.vector.tensor_scalar_add(out=t1, in0=x2[0:BL, :], scalar1=1.0)
            nc.vector.tensor_mul(t1, dotp[0:BL, :], t1)
            t2 = stats.tile([BL, 1], F32, tag="t2")
            nc.vector.tensor_scalar_add(out=t2, in0=u2[0:BL, :], scalar1=1.0)
            nc.vector.tensor_mul(t2, x2[0:BL, :], t2)
            cosn = stats.tile([BL, 1], F32, tag="cosn")
            nc.vector.tensor_sub(cosn, t1, t2)
            rden = stats.tile([BL, 1], F32, tag="rden")
            nc.vector.reciprocal(out=rden, in_=den)
            nc.vector.tensor_mul(cosn, cosn, rden)
            nc.vector.tensor_scalar(
                out=cosn, in0=cosn, scalar1=-BND, scalar2=BND, op0=ALU.max, op1=ALU.min
            )
            c2t = stats.tile([BL, 1], F32, tag="c2t")
            nc.vector.tensor_mul(c2t, cosn, cosn)
            nc.vector.tensor_scalar(
                out=c2t, in0=c2t, scalar1=-1.0, scalar2=1.0, op0=ALU.mult, op1=ALU.add
            )
            sc2 = stats.tile([BL, 1], F32, tag="sc2")
            nc.scalar.activation(out=sc2, in_=c2t, func=AF.Sqrt)
            rsc = stats.tile([BL, 1], F32, tag="rsc")
            nc.vector.reciprocal(out=rsc, in_=sc2)
            aarg = stats.tile([BL, 1], F32, tag="aarg")
            nc.vector.tensor_mul(aarg, cosn, rsc)

            # ---- I: angles = pi/2 - arctan(aarg) ----
            atv = stats.tile([BL, 1], F32, tag="atv")
            nc.scalar.activation(out=atv, in_=aarg, func=AF.Arctan)
            ang = stats.tile([BL, 1], F32, tag="ang")
            nc.vector.tensor_scalar(
                out=ang, in0=atv, scalar1=PI_HALF, scalar2=-1.0,
                op0=ALU.subtract, op1=ALU.mult,
            )

            # ---- J: exp(-dsq) = (1-dn)/(1+dn); dsq = ln((1+dn)/(1-dn)) ----
            opd = stats.tile([ND, 1], F32, tag="opd")
            nc.vector.tensor_scalar_add(out=opd, in0=dn, scalar1=1.0)
            omd = stats.tile([ND, 1], F32, tag="omd")
            nc.vector.tensor_scalar(
                out=omd, in0=dn, scalar1=-1.0, scalar2=1.0, op0=ALU.mult, op1=ALU.add
            )
            ropd = stats.tile([ND, 1], F32, tag="ropd")
            nc.vector.reciprocal(out=ropd, in_=opd)
            en = stats.tile([ND, 1], F32, tag="en")
            nc.vector.tensor_mul(en, omd, ropd)           # exp(-dsq), all 40 pairs
            romd = stats.tile([BL, 1], F32, tag="romd")
            nc.vector.reciprocal(out=romd, in_=omd[0:BL, :])
            ratio = stats.tile([BL, 1], F32, tag="ratio")
            nc.vector.tensor_mul(ratio, opd[0:BL, :], romd)
            dsq = stats.tile([BL, 1], F32, tag="dsq")
            nc.scalar.activation(out=dsq, in_=ratio, func=AF.Ln)

            # Z1 gather (neg pair rows BL + j*BL + b) and final loss rows
            en84 = stats.tile([BL, NEG], F32, tag="en84")
            for jj in range(NEG):
                eng = nc.sync if jj % 2 else nc.gpsimd
                eng.dma_start(
                    out=en84[:, jj : jj + 1],
                    in_=en[BL + jj * BL : BL + (jj + 1) * BL, :],
                )
            z1 = stats.tile([BL, 1], F32, tag="z1")
            nc.vector.reduce_sum(out=z1, in_=en84, axis=mybir.AxisListType.X)
            nc.vector.tensor_add(z1, z1, en[0:BL, :])
            lnz = stats.tile([BL, 1], F32, tag="lnz")
            nc.scalar.activation(out=lnz, in_=z1, func=AF.Ln)
            lrow = stats.tile([BL, 1], F32, tag="lrow")
            nc.vector.tensor_add(lrow, lnz, dsq)
            nc.vector.tensor_add(lrow, lrow, ang)
            nc.sync.dma_start(out=out[:], in_=lrow)

    nc.compile()
    return nc


_NC_CACHE = None


def _get_nc():
    global _NC_CACHE
    if _NC_CACHE is None:
        _NC_CACHE = _build_nc()
    return _NC_CACHE


def _prep_core_inputs(encoded, n_encoded, mask1, mask2, mask_u_neg, W):
    m1 = np.ascontiguousarray(mask1.reshape(B, S), dtype=np.float32)
    m2 = np.ascontiguousarray(mask2.reshape(B, S), dtype=np.float32)
    mnr = np.ascontiguousarray(mask_u_neg.reshape(B * NEG, S), dtype=np.float32)
    wid = np.zeros((128, HC * D + 128), dtype=np.float32)
    wid[:, 0 : HC * D] = (
        W.astype(np.float32).T.reshape(HC, 128, D).transpose(1, 0, 2).reshape(128, -1)
    )
    wid[:, HC * D :] = np.eye(128, dtype=np.float32)
    iota = np.arange(S, dtype=np.float32)
    # selected-row -> local token-table row: u_b -> b, v_b -> b, neg (j-major) -> 8+jl
    rowbase = (
        np.concatenate([np.arange(BL), np.arange(BL), BL + np.arange(NL)]).astype(
            np.float32
        )
        * S
    )
    in_maps = []
    for m in range(NCORES):
        b0, j0 = m * BL, m * NL
        nenc_l = (
            n_encoded[j0 : j0 + NL]
            .reshape(BL, NEG, S, H)
            .transpose(1, 0, 2, 3)
            .reshape(NL, S, H)
        )
        allenc = np.concatenate(
            [np.asarray(encoded[b0 : b0 + BL], dtype=np.float32), nenc_l], axis=0
        )
        mn_l = (
            mnr[j0 : j0 + NL].reshape(BL, NEG, S).transpose(1, 0, 2).reshape(NL, S)
        )
        mall = np.concatenate([m1[b0 : b0 + BL], m2[b0 : b0 + BL], mn_l], axis=0)
        mio = np.concatenate(
            [mall, np.broadcast_to(iota, (NP, S)), rowbase[:, None]], axis=1
        )
        in_maps.append(
            {
                "allenc": np.ascontiguousarray(allenc),
                "mio": np.ascontiguousarray(mio, dtype=np.float32),
                "wid": wid,
            }
        )
    return in_maps


def kernel(encoded, n_encoded, mask1, mask2, mask_u_neg, W):
    nc = _get_nc()
    in_maps = _prep_core_inputs(encoded, n_encoded, mask1, mask2, mask_u_neg, W)
    res = run_bass_kernel_spmd(nc, in_maps, core_ids=list(range(NCORES)))
    rows = np.concatenate([r["out"][:, 0] for r in res.results])
    return np.float32(rows.mean())
